# revision 20
# baseline (speedup 1.0000x reference)
"""Dense MoE layer (8 experts, all-expert weighted combine) on 8 TRN2 NeuronCores.

Strategy: data-parallel over the token dim. Each core gets a 1024-token shard
(pre-transposed + bf16-cast on host), the full stacked expert weights (bf16),
and computes gate softmax + all 8 expert matmuls + gate-weighted combine
locally. No collectives; host concatenates the 8 output shards.

Per-core device schedule (software-pipelined over token tiles t):
  - y(e,t) = x_t @ We[e]: 16 matmuls (8 K-chunks x 2 N=512 halves) accumulate
    in PSUM f32; combine is one fused DVE op: out = psum*g[:,e] + out.
  - gate logits are computed TRANSPOSED (lhsT = Wg chunk, 8-column weight
    loads are ~free) into [8,128] psum, bias-added as a per-partition scalar,
    then PE-transposed back to [128,8] for the free-dim softmax. exp uses
    accum_out to produce the softmax denominator for free.
  - the bias term g@be is a K=8 matmul against the PE-transposed gate.
  - pipeline: block t runs y(0,t) while gate(t+1) is prefetched, softmax(t)
    resolves, g(t-1) transposes, and bias(t-2) lands. Keeps PE gap-free.

DMA issue costs ~0.6us per dma_start on the issuing engine, so issue is
spread: gpsimd (first xT chunks + identities), sync (rest of xT + outputs),
vector (small gate constants), scalar (all We chunks, double-buffered).
"""

import os
import sys

import numpy as np

try:
    import concourse.bass as bass  # noqa: F401
except ImportError:  # harness containers stage the repo at /opt/trn_rl_repo
    sys.path.insert(0, "/opt/trn_rl_repo")

from contextlib import ExitStack

import ml_dtypes

import concourse.bass as bass
import concourse.mybir as mybir
import concourse.tile as tile
from concourse import bacc
from concourse.bass_utils import run_bass_kernel_spmd

N_CORES = 8
N_TOK = 8192
IN_F = 1024
OUT_F = 1024
E = 8
P = 128  # partitions


def build_nc(n_tok_pc: int = N_TOK // N_CORES, debug: bool = False):
    """Build the single-core SPMD Bass program (same program on all 8 cores)."""
    fp32 = mybir.dt.float32
    bf16 = mybir.dt.bfloat16

    K_CH = IN_F // P  # contraction chunks of 128
    T = n_tok_pc // P  # token tiles per core
    assert T >= 2

    nc = bacc.Bacc(
        "TRN2", target_bir_lowering=False, debug=debug, enable_asserts=False
    )

    xT = nc.declare_dram_parameter("xT", [IN_F, n_tok_pc], bf16, isOutput=False)
    We = nc.declare_dram_parameter("We", [E, IN_F, OUT_F], bf16, isOutput=False)
    be = nc.declare_dram_parameter("be", [E, OUT_F], bf16, isOutput=False)
    Wg = nc.declare_dram_parameter("Wg", [P, K_CH, E], bf16, isOutput=False)
    bgc = nc.declare_dram_parameter("bgc", [E, 1], fp32, isOutput=False)
    idn = nc.declare_dram_parameter("idn", [P, P], fp32, isOutput=False)
    out = nc.declare_dram_parameter("out", [n_tok_pc, OUT_F], fp32, isOutput=True)

    with tile.TileContext(nc) as tc, ExitStack() as ctx:
        consts = ctx.enter_context(tc.tile_pool(name="consts", bufs=1))
        xpool = ctx.enter_context(tc.tile_pool(name="xpool", bufs=1))
        wepool = ctx.enter_context(tc.tile_pool(name="wepool", bufs=2))
        opool = ctx.enter_context(tc.tile_pool(name="opool", bufs=1))
        gpool = ctx.enter_context(tc.tile_pool(name="gpool", bufs=1))
        small = ctx.enter_context(tc.tile_pool(name="small", bufs=4))
        # 8 PSUM banks: 3 x [128,1024] f32 (2 banks each) + 2 shared small slots
        psum_y = ctx.enter_context(tc.tile_pool(name="psum_y", bufs=3, space="PSUM"))
        psum_g = ctx.enter_context(tc.tile_pool(name="psum_g", bufs=2, space="PSUM"))

        # ---- input DMAs spread across engines for issue throughput ----
        # xT chunks on sync: the PE-critical path
        xT_sb = []
        for c in range(K_CH):
            xc = xpool.tile([P, n_tok_pc], bf16, tag=f"xt{c}")
            nc.sync.dma_start(out=xc, in_=xT[c * P : (c + 1) * P, :])
            xT_sb.append(xc)

        # small constants ride gpsimd's (software) DGE — fast for small
        # transfers and keeps the sync/scalar issue streams free for the bulk
        wg_sb = consts.tile([P, K_CH, E], bf16)
        nc.gpsimd.dma_start(out=wg_sb, in_=Wg[:, :, :])
        ident = consts.tile([P, P], fp32)
        nc.gpsimd.dma_start(out=ident, in_=idn[:, :])
        ident8 = ident[:E, :E]
        bgc_sb = consts.tile([E, 1], fp32)
        nc.gpsimd.dma_start(out=bgc_sb, in_=bgc[:, :])
        be_sb = consts.tile([E, OUT_F], bf16)
        nc.gpsimd.dma_start(out=be_sb, in_=be[:, :])

        def fetch_we_chunk(e, c):
            wc = wepool.tile([P, OUT_F], bf16, tag=f"we{c}")
            nc.scalar.dma_start(out=wc, in_=We[e, c * P : (c + 1) * P, :])
            return wc

        def fetch_we(e):
            return [fetch_we_chunk(e, c) for c in range(K_CH)]

        we_sb = {0: fetch_we(0)}

        # HAM warmup: the PE is idle ~3.5us waiting for the first input DMAs;
        # dummy matmuls on a memset tile during that window un-throttle the
        # clock gate (1.2 -> 2.4 GHz) before the real stream begins.
        warm_sb = consts.tile([P, 512], bf16)
        nc.vector.memset(warm_sb, 0.25)
        wps = psum_g.tile([P, 512], fp32, tag="g8")
        for _ in range(16):
            nc.tensor.matmul(
                wps, lhsT=warm_sb[:, 0:P], rhs=warm_sb, start=True, stop=True
            )

        g_sb = gpool.tile([P, T, E], fp32)
        gT_sb = gpool.tile([E, T, P], bf16)
        lgT_sb = gpool.tile([E, T, P], fp32)
        out_sb = opool.tile([P, T, OUT_F], fp32)

        def main_mms(e, t):
            py = psum_y.tile([P, OUT_F], fp32, tag="y")
            tok = slice(t * P, (t + 1) * P)
            for c in range(K_CH):
                for h in range(2):
                    hs = slice(h * 512, (h + 1) * 512)
                    nc.tensor.matmul(
                        py[:, hs],
                        lhsT=xT_sb[c][:, tok],
                        rhs=we_sb[e][c][:, hs],
                        start=(c == 0),
                        stop=(c == K_CH - 1),
                    )
            return py

        def gate_mms(t):
            # transposed gate logits: lhsT = Wg chunk (8-col weight load)
            tok = slice(t * P, (t + 1) * P)
            lgt = psum_g.tile([E, P], fp32, tag="g8")
            for c in range(K_CH):
                nc.tensor.matmul(
                    lgt,
                    lhsT=wg_sb[:, c, :],
                    rhs=xT_sb[c][:, tok],
                    start=(c == 0),
                    stop=(c == K_CH - 1),
                )
            # += bg (per-partition scalar in transposed space)
            nc.vector.tensor_scalar_add(lgt, lgt, bgc_sb[:, :])
            nc.scalar.copy(out=lgT_sb[:, t, :], in_=lgt)

        def softmax(t):
            # PE-transpose logits back to [tok, e], then free-dim softmax
            lg = psum_g.tile([P, E], fp32, tag="g8")
            nc.tensor.transpose(lg, lgT_sb[:, t, :], ident8)
            neg_m = small.tile([P, 1], fp32, tag="negm")
            nc.vector.reduce_max(
                out=neg_m, in_=lg, axis=mybir.AxisListType.X, negate=True
            )
            gexp = small.tile([P, E], fp32, tag="gexp")
            ssum = small.tile([P, 1], fp32, tag="ssum")
            nc.scalar.activation(
                out=gexp,
                in_=lg,
                func=mybir.ActivationFunctionType.Exp,
                bias=neg_m,
                scale=1.0,
                accum_out=ssum,
            )
            rsum = small.tile([P, 1], fp32, tag="rsum")
            nc.vector.reciprocal(out=rsum, in_=ssum)
            nc.vector.tensor_scalar_mul(g_sb[:, t, :], gexp, rsum)

        def transpose_g(t):
            gt = psum_g.tile([E, P], fp32, tag="g8")
            nc.tensor.transpose(gt, g_sb[:, t, :], ident)
            nc.scalar.copy(out=gT_sb[:, t, :], in_=gt)

        def combine0(t, py):
            # out[t] = y(e=0) * g[:, 0]   (overwrite-init; bias added later)
            nc.vector.tensor_scalar_mul(
                out_sb[:, t, :], py[:, :], g_sb[:, t, 0:1]
            )

        def bias_mms(t):
            pb = psum_y.tile([P, OUT_F], fp32, tag="y")
            for h in range(2):
                hs = slice(h * 512, (h + 1) * 512)
                nc.tensor.matmul(
                    pb[:, hs], lhsT=gT_sb[:, t, :], rhs=be_sb[:, hs],
                    start=True, stop=True,
                )
            # out[t] += g @ be
            nc.vector.tensor_tensor(
                out=out_sb[:, t, :],
                in0=pb[:, :],
                in1=out_sb[:, t, :],
                op=mybir.AluOpType.add,
            )

        def combine(e, t, py):
            # out[t] = y(e) * g[:, e] + out[t]   (fused on DVE)
            nc.vector.scalar_tensor_tensor(
                out=out_sb[:, t, :],
                in0=py[:, :],
                scalar=g_sb[:, t, e : e + 1],
                in1=out_sb[:, t, :],
                op0=mybir.AluOpType.mult,
                op1=mybir.AluOpType.add,
            )

        # ---- phase A: e=0 pipelined with the gate computation ----
        # block t emission order is chosen so every PE op's dependencies
        # resolved at least one block earlier (no PE stalls):
        #   main(0,t) | transpose g(t-1) | combine0(t-1) | gate mms(t+1) |
        #   softmax(t) | bias(t-2)
        py_live = {}
        gate_mms(0)
        # We[1] issue is spread across phase-A blocks so the gate chain's ACT
        # ops aren't starved by a burst of dma_start issue time on scalar
        cpb = (K_CH + min(T, K_CH) - 1) // min(T, K_CH)
        we_sb[1] = []
        for t in range(T):
            py_live[t] = main_mms(0, t)
            if t >= 1:
                transpose_g(t - 1)
                combine0(t - 1, py_live.pop(t - 1))
            if t + 1 < T:
                gate_mms(t + 1)
            softmax(t)
            while len(we_sb[1]) < min(K_CH, (t + 1) * cpb):
                we_sb[1].append(fetch_we_chunk(1, len(we_sb[1])))
            if t >= 2:
                bias_mms(t - 2)

        # bridge: keep PE fed while softmax(T-1) resolves
        py_b = main_mms(1, 0)

        transpose_g(T - 1)
        combine0(T - 1, py_live.pop(T - 1))
        bias_mms(T - 2)
        bias_mms(T - 1)
        combine(1, 0, py_b)

        # ---- phase B: experts 1..7 ----
        for e in range(1, E):
            if e + 1 < E:
                we_sb[e + 1] = fetch_we(e + 1)
            for t in range(1 if e == 1 else 0, T):
                py = main_mms(e, t)
                if e == E - 1 and t == T - 1:
                    # final tile: combine + write back in halves so the last
                    # out DMA overlaps the last combine
                    for h in range(2):
                        hs = slice(h * 512, (h + 1) * 512)
                        nc.vector.scalar_tensor_tensor(
                            out=out_sb[:, t, hs],
                            in0=py[:, hs],
                            scalar=g_sb[:, t, e : e + 1],
                            in1=out_sb[:, t, hs],
                            op0=mybir.AluOpType.mult,
                            op1=mybir.AluOpType.add,
                        )
                        nc.sync.dma_start(
                            out=out[t * P : (t + 1) * P, hs],
                            in_=out_sb[:, t, hs],
                        )
                else:
                    combine(e, t, py)
                    if e == E - 1:
                        # write back this tile right after its final combine
                        nc.sync.dma_start(
                            out=out[t * P : (t + 1) * P, :], in_=out_sb[:, t, :]
                        )
            del we_sb[e - 1]

    nc.compile()
    return nc


_NC_CACHE: dict = {}


def _get_nc(n_tok_pc: int):
    if n_tok_pc not in _NC_CACHE:
        _NC_CACHE[n_tok_pc] = build_nc(n_tok_pc)
    return _NC_CACHE[n_tok_pc]


def make_in_maps(x, We, be, Wg, bg):
    """Host-side sharding: token-shard + transpose x, bf16-cast everything."""
    bf16 = ml_dtypes.bfloat16
    x = np.asarray(x)
    n_tok_pc = x.shape[0] // N_CORES
    We_bf = np.asarray(We).astype(bf16)
    be_bf = np.asarray(be).astype(bf16)
    K_CH = IN_F // P
    # [1024, 8] -> [p, chunk, e]
    Wg_bf = (
        np.asarray(Wg).astype(bf16).reshape(K_CH, P, E).transpose(1, 0, 2).copy()
    )
    bg_col = np.asarray(bg).astype(np.float32).reshape(E, 1)
    ident = np.eye(P, dtype=np.float32)
    xbf = x.astype(bf16)
    in_maps = []
    for cid in range(N_CORES):
        xs = xbf[cid * n_tok_pc : (cid + 1) * n_tok_pc]
        in_maps.append(
            {
                "xT": np.ascontiguousarray(xs.T),
                "We": We_bf,
                "be": be_bf,
                "Wg": Wg_bf,
                "bgc": bg_col,
                "idn": ident,
            }
        )
    return in_maps, n_tok_pc


def run(x, We, be, Wg, bg, trace=False, **trace_kwargs):
    in_maps, n_tok_pc = make_in_maps(x, We, be, Wg, bg)
    nc = _get_nc(n_tok_pc)
    res = run_bass_kernel_spmd(
        nc, in_maps, core_ids=list(range(N_CORES)), trace=trace, **trace_kwargs
    )
    outs = [res.results[i]["out"] for i in range(N_CORES)]
    return np.concatenate(outs, axis=0), res


def kernel(x, We, be, Wg, bg):
    out, _ = run(x, We, be, Wg, bg, trace=False)
    return out
